# revision 48
# baseline (speedup 1.0000x reference)
"""CliffordLinear (Cl(3,0)) Trainium2 kernel — Karatsuba complex matmul, fp16.

Math: Cl(3,0) ~ 2x2 complex matrices via Pauli rep phi.  The reference
out[b,o] = sum_i W[o,i] * X[b,i] (Clifford product) maps to
OutM[b,o] = sum_i phi(W[o,i]) @ phi(X[b,i]).  Per output column c of phi(X),
this is a complex matmul Out_c[b,(o,r)] = X_c[b,(i,m)] @ Wc[(i,m),(o,r)]
with Wc = R + iI shared by both c.  Karatsuba (3 real matmuls per complex
matmul instead of 4):

    M1 = XRe_c @ R;  M2 = XIm_c @ I;  M3 = (XRe_c + XIm_c) @ (R + I)
    OutRe_c = M1 - M2;   OutIm_c = M3 - M1 - M2

12.9G real MACs total (vs 17.2G for the 4-matmul split, 34.4G naive blade).
All matmul operands fp16 (PSUM accumulates f32): same PE rate as fp32r but
half the DMA traffic.  The Karatsuba sum operand ships pre-computed from the
host (DMA has slack; DVE does not).  PSUM results are evicted to fp16 SBUF
by the ACT engine; the Karatsuba combines and the inverse-Pauli butterfly
run on DVE in fp16 2x mode (all-SBUF, packed stride-1 last dims — weight
columns are laid out r-major and output blade-major to keep them packed).
Host undoes the blade-major layout with one transpose.

Schedule notes (PE is the roofline at ~41 us busy; everything else hides
under it): M3 gets double-buffered PSUM banks because the tile framework
releases an evicted bank ~6 matmuls later than its data dependency;
steady-state stores go through the otherwise-idle Pool/SWDGE queue so the
ACT queue never blocks an eviction.  Last b-tile: c1's M3 accumulation
group is extended with two negated-identity matmuls over the evicted
m1|m2, so its PSUM bank holds Im = M3-M1-M2 directly (PE is free at the
tail; this removes two serial DVE combines); the Re-blade ops + stores are
issued ahead of that final chain, and the Im blades are two merged dual
ops feeding separate ACT/SP HWDGE stores (Pool's ~1 us SWDGE descriptor
gen would serialize the tail).

Sharding: data-parallel over batch (1024 rows/core); weights replicated.
Per-core HBM: 6 MB x + 1.5 MB w in, 4 MB out (~33 us DMA vs ~41 us PE).
"""

import sys

sys.path.insert(0, "/opt/trn_rl_repo")

import numpy as np

import concourse.bass as bass  # noqa: F401  (registers lowerings)
import concourse.mybir as mybir
import concourse.tile as tile
from concourse import bacc
from concourse.bass_utils import run_bass_kernel_spmd

N_CORES = 8
B, CIN, COUT, NB = 8192, 256, 256, 8
BS = B // N_CORES          # 1024 batch rows per core
HK = 512                   # contraction rows per Re/Im half: (i, m)
XW = 12 * 128              # x tile width: [Re k0-3 | Im k0-3 | Sum k0-3] x 128b
OUTW = COUT * NB           # 2048 output cols, BLADE-major: col = l*256 + o
BT = BS // 128             # 8 b-tiles

_cached = {}


def _ap3(base, off, dims):
    """Rewrite base's free AP to `dims` (list of (stride, count)), at
    element offset `off` into the tile row."""
    a = base.copy()
    part = a.ap.to_list()[0]
    v = a.ap
    v.clear()
    v.extend([tuple(part)] + [tuple(d) for d in dims])
    a.offset = a.offset + off
    return a


def _build_nc():
    f16 = mybir.dt.float16
    f32 = mybir.dt.float32
    nc = bacc.Bacc("TRN2", target_bir_lowering=False, debug=False,
                   num_devices=N_CORES)
    # x per column c: [bt, p, j, b]; row j*128+p: j 0-3 XRe k-tiles,
    # 4-7 XIm, 8-11 XSum (host-side XRe+XIm).
    xt0 = nc.dram_tensor("xt0", [BT, 128, XW], f16, kind="ExternalInput")
    xt1 = nc.dram_tensor("xt1", [BT, 128, XW], f16, kind="ExternalInput")
    # weight planes [R, I, R+I]: [(i,m), (r,o)] — columns r-major, rows
    # pre-tiled host-side to [p(128), k(4), col(512)] for a 1-trigger DMA.
    wri = nc.dram_tensor("wri", [3, 128, 4 * HK], f16, kind="ExternalInput")
    # negated 128x128 identity: lets the PE fold "- M1 - M2" into the last
    # b-tile's M3 accumulation group (tail shortening).
    negid = nc.dram_tensor("negid", [128, 128], f16, kind="ExternalInput")
    out = nc.dram_tensor("out", [BS, OUTW], f16, kind="ExternalOutput")

    with tile.TileContext(nc) as tc:
        with tc.tile_pool(name="wpool", bufs=1) as wpool, \
             tc.tile_pool(name="xpool", bufs=3) as xpool, \
             tc.tile_pool(name="epool", bufs=3) as epool, \
             tc.tile_pool(name="opool", bufs=3) as opool, \
             tc.tile_pool(name="pspool", bufs=1, space="PSUM") as pspool:
            # PE warmup: ramp the clock gate during the initial DMA wait,
            # sized to finish right as the first weight/x DMAs land.
            # Warmup accumulates into an m3_0-tagged PSUM buffer (no spare
            # bank: m12 tiles 2x2 banks + m3 tiles 2x2 banks = all 8).
            warm_in = wpool.tile([128, 640], mybir.dt.bfloat16, tag="warm_in")
            nc.vector.memset(warm_in[:], 0.0)
            warm_ps = pspool.tile([128, 512], f32, tag="m3_0", bufs=2,
                                  name="warm_ps")
            for _ in range(8):
                nc.tensor.matmul(warm_ps[:], warm_in[:, :128], warm_in[:, 128:640],
                                 start=True, stop=True)

            # Weight planes in SBUF: [128, k(4) x 512].
            wt = [wpool.tile([128, 4 * HK], f16, tag=f"w{p}", name=f"w{p}")
                  for p in range(3)]

            def load_w(p):
                nc.sync.dma_start(wt[p][:], wri[p])

            xt = [xt0, xt1]

            def load_x(bt, c, sum_now=True):
                xc = xpool.tile([128, XW], f16, tag=f"x{c}", name=f"x{c}")
                # Re+Im sections, then Sum section (separate DMAs so the
                # matmuls can start before the Sum lands).
                nc.sync.dma_start(xc[:, 0:1024], xt[c][bt][:, 0:1024])
                if sum_now:
                    load_x_sum(bt, c, xc)
                return xc

            def load_x_sum(bt, c, xc):
                nc.sync.dma_start(xc[:, 1024:1536], xt[c][bt][:, 1024:1536])

            # Startup order interleaves weight planes with bt0's x so the
            # PE never waits: R, x0[RI], I, x1[RI], R+I, x0[S], x1[S].
            load_w(0)
            x_cur = [None, None]
            x_cur[0] = load_x(0, 0, sum_now=False)
            load_w(1)
            x_cur[1] = load_x(0, 1, sum_now=False)
            load_w(2)
            load_x_sum(0, 0, x_cur[0])
            load_x_sum(0, 1, x_cur[1])
            nid = wpool.tile([128, 128], f16, tag="nid")
            nc.scalar.dma_start(nid[:], negid[:, :])

            m12 = [pspool.tile([128, 2 * HK], f32, tag=f"m12_{c}", name=f"m12_{c}")
                   for c in (0, 1)]
            m3s = {}

            def mm_group(c, g):
                """g: 0 = M1 (XRe@R), 1 = M2 (XIm@I), 2 = M3 (XSum@(R+I)).
                M3 gets its own double-buffered bank so the (late) M3 evict
                never gates the next b-tile's matmuls."""
                xc, w = x_cur[c], wt[g]
                if g == 2:
                    t = pspool.tile([128, HK], f32, tag=f"m3_{c}", bufs=2,
                                    name=f"m3_{c}")
                    m3s[c] = t
                    p = t[:]
                else:
                    p = m12[c][:, g * HK:(g + 1) * HK]
                for k in range(4):
                    nc.tensor.matmul(p,
                                     xc[:, (g * 4 + k) * 128:(g * 4 + k + 1) * 128],
                                     w[:, k * HK:(k + 1) * HK],
                                     start=(k == 0), stop=(k == 3))

            def combine12(c):
                """Evict M1|M2 to fp16 SBUF (ACT) and form D = M1-M2 (DVE).
                Issue right after (c,1) so ACT/DVE trail the PE closely."""
                e = epool.tile([128, 3 * HK], f16, tag=f"e{c}", name=f"e{c}")
                nc.scalar.copy(e[:, 0:1024], m12[c][:])
                d = epool.tile([128, 2 * HK], f16, tag=f"di{c}", name=f"di{c}")
                nc.vector.tensor_sub(d[:, 0:HK], e[:, 0:HK], e[:, HK:1024])
                return e, d

            def combine3(c, e, d):
                """Evict M3; I = (M3 - M1) - M2 into d's upper half."""
                nc.scalar.copy(e[:, 1024:1536], m3s[c][:])
                u = epool.tile([128, HK], f16, tag=f"u{c}", name=f"u{c}")
                nc.vector.tensor_sub(u[:], e[:, 1024:1536], e[:, 0:HK])
                nc.vector.tensor_sub(d[:, HK:1024], u[:], e[:, HK:1024])

            # Inverse Pauli butterfly, blade-major output.  DI layout
            # [D(r0)|D(r1)|I(r0)|I(r1)] x 256; j toggles Re/Im blade:
            #   x0,x7 = P[r0] + Q[r1]   x4,x3 = P[r0] - Q[r1]
            #   x1,x6 = P[r1] + Q[r0]   x5,x2 = P[r1] - Q[r0]
            O = 256
            BFLY = [(0, 7, 0, 1, True), (4, 3, 0, 1, False),
                    (1, 6, 1, 0, True), (5, 2, 1, 0, False)]

            def bfly(stage, P, Q):
                add, sub = nc.vector.tensor_add, nc.vector.tensor_sub
                for lre, lim, rp, rq, is_add in BFLY:
                    op = add if is_add else sub
                    op(_ap3(stage[:], lre * O, [((lim - lre) * O, 2), (1, O)]),
                       _ap3(P[:], rp * O, [(2 * O, 2), (1, O)]),
                       _ap3(Q[:], rq * O, [(2 * O, 2), (1, O)]))

            for bt in range(BT):
                last = bt == BT - 1
                if bt == 0:
                    # M3 groups last: their weights/x-sum arrive last.
                    order = [(0, 0), (0, 1), (1, 0), (1, 1), (0, 2), (1, 2)]
                elif last:
                    # c1's M3 last (issued inline below with the -M1-M2
                    # fold); everything c0 completes early so only c1's Im
                    # chain trails the final matmul.
                    order = [(1, 0), (1, 1), (0, 0), (0, 1), (0, 2)]
                else:
                    order = [(0, 0), (0, 1), (0, 2), (1, 0), (1, 1), (1, 2)]
                x_next = [None, None]
                if not last:
                    x_next[0] = load_x(bt + 1, 0)
                    x_next[1] = load_x(bt + 1, 1)
                ed = {}
                for c, g in order:
                    mm_group(c, g)
                    if g == 1:
                        ed[c] = combine12(c)
                    elif g == 2 and not (last and c == 1):
                        combine3(c, *ed[c])
                P, Q = ed[0][1], ed[1][1]
                stage = opool.tile([128, OUTW], f16, tag="stage")
                orow = out[bt * 128:(bt + 1) * 128]
                if not last:
                    bfly(stage, P, Q)
                    nc.gpsimd.dma_start(orow, stage[:])
                else:
                    # Tail.  c1's M3 group is extended with two negated-
                    # identity matmuls over the evicted m1|m2, so the PSUM
                    # bank holds I_1 = M3-M1-M2 directly (PE is free here;
                    # this removes two serial DVE combines from the tail).
                    e1, d1 = ed[1]
                    t = pspool.tile([128, HK], f32, tag="m3_1", bufs=2,
                                    name="m3_1t")
                    xc1 = x_cur[1]
                    for k in range(4):
                        nc.tensor.matmul(t[:],
                                         xc1[:, (8 + k) * 128:(9 + k) * 128],
                                         wt[2][:, k * HK:(k + 1) * HK],
                                         start=(k == 0), stop=False)
                    nc.tensor.matmul(t[:], nid[:], e1[:, 0:HK],
                                     start=False, stop=False)
                    nc.tensor.matmul(t[:], nid[:], e1[:, HK:1024],
                                     start=False, stop=True)
                    # Re blades need only the D halves, ready before the
                    # final M3 — two merged dual ops (j = r-pair) + Pool
                    # stores, issued ahead of c1's final chain so the
                    # in-order DVE queue doesn't park them.
                    add, sub = nc.vector.tensor_add, nc.vector.tensor_sub
                    add(_ap3(stage[:], 0, [(O, 2), (1, O)]),        # x0, x1
                        _ap3(P[:], 0, [(O, 2), (1, O)]),
                        _ap3(Q[:], O, [(-O, 2), (1, O)]))
                    sub(_ap3(stage[:], 4 * O, [(O, 2), (1, O)]),    # x4, x5
                        _ap3(P[:], 0, [(O, 2), (1, O)]),
                        _ap3(Q[:], O, [(-O, 2), (1, O)]))
                    # All last-tile stores via fast HWDGE queues (Pool's
                    # SWDGE gen is ~1 us each and would serialize the tail).
                    nc.sync.dma_start(orow[:, 0:2 * O], stage[:, 0:2 * O])
                    nc.sync.dma_start(orow[:, 4 * O:6 * O], stage[:, 4 * O:6 * O])
                    # ACT evicts I_1 straight into the butterfly input tile,
                    # then merged dual Im ops, each feeding its own store.
                    nc.scalar.copy(d1[:, HK:1024], t[:])
                    sub(_ap3(stage[:], 3 * O, [(-O, 2), (1, O)]),   # x3, x2
                        _ap3(P[:], 2 * O, [(O, 2), (1, O)]),
                        _ap3(Q[:], 3 * O, [(-O, 2), (1, O)]))
                    nc.scalar.dma_start(orow[:, 2 * O:4 * O],
                                        stage[:, 2 * O:4 * O])
                    add(_ap3(stage[:], 7 * O, [(-O, 2), (1, O)]),   # x7, x6
                        _ap3(P[:], 2 * O, [(O, 2), (1, O)]),
                        _ap3(Q[:], 3 * O, [(-O, 2), (1, O)]))
                    nc.sync.dma_start(orow[:, 6 * O:8 * O],
                                      stage[:, 6 * O:8 * O])
                x_cur = x_next
    nc.finalize()
    return nc


def _pauli_parts(v):
    """v[..., 8] -> c0, c1 of shape [..., 2, 2]: the c-th column (rows, reim)
    of phi(v).  phi = [[A, B], [C, D]]: A=(v0+v4)+i(v3+v7), B=(v1-v5)+i(v6-v2),
    C=(v1+v5)+i(v6+v2), D=(v0-v4)+i(v7-v3)."""
    c0 = np.empty(v.shape[:-1] + (2, 2), dtype=v.dtype)
    c1 = np.empty_like(c0)
    v0, v1, v2, v3, v4, v5, v6, v7 = (v[..., a] for a in range(8))
    c0[..., 0, 0] = v0 + v4   # Re A
    c0[..., 0, 1] = v3 + v7   # Im A
    c0[..., 1, 0] = v1 + v5   # Re C
    c0[..., 1, 1] = v6 + v2   # Im C
    c1[..., 0, 0] = v1 - v5   # Re B
    c1[..., 0, 1] = v6 - v2   # Im B
    c1[..., 1, 0] = v0 - v4   # Re D
    c1[..., 1, 1] = v7 - v3   # Im D
    return c0, c1


def _prep_w(weight):
    """weight [COUT, CIN, 8] -> [3, 512, 512] fp16 planes R | I | R+I of
    phi(W)[r,m] indexed [(i,m), (r,o)] (columns r-major), 0.5 inverse-
    butterfly factor folded in."""
    w = weight.astype(np.float32)
    cw0, cw1 = _pauli_parts(w)     # cw_m[o,i,r,reim] = phi(W[o,i])[r,m]
    R = np.empty((CIN, 2, 2, COUT), np.float32)   # [i, m, r, o]
    I = np.empty_like(R)
    for m, cm in ((0, cw0), (1, cw1)):
        R[:, m] = 0.5 * cm[:, :, :, 0].transpose(1, 2, 0)
        I[:, m] = 0.5 * cm[:, :, :, 1].transpose(1, 2, 0)
    R = R.reshape(HK, HK)
    I = I.reshape(HK, HK)
    planes = np.stack([R, I, R + I])                  # [3, (i,m), (r,o)]
    # rows (i,m) -> [p(128), k(4)] tiling: plane[:, k*128+p, :] lands at
    # SBUF partition p, column block k.
    planes = planes.reshape(3, 4, 128, HK).transpose(0, 2, 1, 3)
    return np.ascontiguousarray(
        planes.reshape(3, 128, 4 * HK)).astype(np.float16)


def _prep_x(x):
    """x [B, CIN, 8] -> per-core fp16 arrays [N_CORES][BT, 128, 12*128] for
    c=0 and c=1: rows j*128+p = [XRe (i,m) | XIm (i,m) | XRe+XIm (i,m)]."""
    xf = x.astype(np.float32)
    c0, c1 = _pauli_parts(xf)          # [B, CIN, m, reim]
    outs = []
    for arr in (c0, c1):
        kb = arr.transpose(3, 1, 2, 0).reshape(2 * HK, B)  # [reim,i,m,b]
        full = np.concatenate([kb, kb[0:HK] + kb[HK:2 * HK]], axis=0)
        a = full.reshape(12, 128, N_CORES, BT, 128)        # [j, p, core, bt, b]
        a = a.transpose(2, 3, 1, 0, 4)                     # [core, bt, p, j, b]
        outs.append(np.ascontiguousarray(
            a.reshape(N_CORES, BT, 128, XW)).astype(np.float16))
    return outs


def kernel(x, weight, bias, cayley):
    assert x.shape == (B, CIN, NB) and weight.shape == (COUT, CIN, NB)
    if "nc" not in _cached:
        _cached["nc"] = _build_nc()
    nc = _cached["nc"]

    xt0, xt1 = _prep_x(np.asarray(x))
    wri = _prep_w(np.asarray(weight))
    nid = (-np.eye(128)).astype(np.float16)
    in_maps = [{"xt0": xt0[c], "xt1": xt1[c], "wri": wri, "negid": nid}
               for c in range(N_CORES)]
    res = run_bass_kernel_spmd(nc, in_maps, core_ids=list(range(N_CORES)))
    o = np.concatenate([res.results[c]["out"] for c in range(N_CORES)], axis=0)
    # blade-major [B, l, o] -> [B, o_channel, blade]
    o = o.reshape(B, NB, COUT).transpose(0, 2, 1).astype(np.float32)
    return o + np.asarray(bias, np.float32)[None]
